# revision 3
# baseline (speedup 1.0000x reference)
"""Grok1-style MoE (T=2048, H=1024, E=8, I=2048, top-2) on 8 Trainium2 cores.

Strategy (expert-parallel, per the sharding hint):
  - Host: compute the tiny router (x @ gate_w, tanh softcap, top-2, softmax)
    and dispatch tokens by expert assignment (the "all-to-all dispatch" step:
    with full inputs on the host, dispatch = gather per expert), packing the
    per-core shards in the device-friendly tiled layout.
  - Device (SPMD, 1 expert per core), all-bf16 matmuls (enables the PE's
    fast-weight-load path and halves HBM traffic; fp32 PSUM accumulate):
      Phase 1:  gT/uT[i, m] = wg_e^T x_e^T / wu_e^T x_e^T  per 128-i-tile
                act[i, m]   = gelu_tanh(gT) * uT            (stored bf16)
        run as two m-chunk sweeps so the first sweep only needs half of xt
        up front; all weight tiles stay resident in SBUF.
      Phase 2:  yT[h, m]    = sum_i wd_e[i, h] * act[i, m]  (exact-M moving
                dim -- no 128-token quantization of the output pass); the
        last h-tile runs in quarter-size m chunks to shorten the output tail.
  - Host: combine = out[ids] += prob * yT.T per expert (prob scaling is
    linear in the expert output, so it folds into the host-side combine).
"""

import numpy as np
import ml_dtypes

import concourse.mybir as mybir
import concourse.tile as tile
from concourse import bacc
from concourse.bass_utils import run_bass_kernel_spmd

T, H, E, I_DIM, TOPK = 2048, 1024, 8, 2048, 2
SOFTCAP = 30.0
P = 128
N_CORES = 8
KH = H // P      # 8 contraction tiles (phase 1)
NI = I_DIM // P  # 16 i tiles
HT = H // P      # 8 h tiles (phase 2 output partitions)

BF16 = ml_dtypes.bfloat16

_compiled = {}
LAST_RESULTS = None


def _build(M_PAD):
    c0 = M_PAD // 2
    chunks = ((0, c0), (c0, M_PAD - c0))
    q = M_PAD // 4
    quarters = ((0, q), (q, q), (2 * q, q), (3 * q, M_PAD - 3 * q))
    f32 = mybir.dt.float32
    bf16 = mybir.dt.bfloat16

    nc = bacc.Bacc("TRN2", target_bir_lowering=False, num_devices=N_CORES)
    # Host-packed layouts (all DMAs contiguous per partition):
    #   xt  [KH, P, M_PAD]   : xt[k, p, m] = x_e[m, k*P+p]
    #   wg  [NI, P, KH*P]    : wg[it, p, k*P+i] = wg_e[k*P+p, it*P+i]
    #   wu  same as wg
    #   wd  [I, H]           : natural layout (row-tile slices are contiguous)
    #   y   [HT, P, M_PAD]   : y[ht, p, m] = yT_e[ht*P+p, m]
    xt = nc.dram_tensor("xt", [KH, P, M_PAD], bf16, kind="ExternalInput")
    wg = nc.dram_tensor("wg", [NI, P, KH * P], bf16, kind="ExternalInput")
    wu = nc.dram_tensor("wu", [NI, P, KH * P], bf16, kind="ExternalInput")
    wd = nc.dram_tensor("wd", [I_DIM, H], bf16, kind="ExternalInput")
    y = nc.dram_tensor("y", [HT, P, M_PAD], bf16, kind="ExternalOutput")

    with tile.TileContext(nc) as tc:
        with (
            tc.tile_pool(name="persist", bufs=1) as persist,
            tc.tile_pool(name="youts", bufs=2) as youts,
            tc.tile_pool(name="psA", bufs=2, space="PSUM") as psA,
            tc.tile_pool(name="psB", bufs=2, space="PSUM") as psB,
        ):
            xt_sb = persist.tile([P, KH, M_PAD], bf16)
            wd_sb = persist.tile([P, NI, H], bf16)
            acts = persist.tile([P, NI, M_PAD], bf16)
            wg_sb = persist.tile([P, NI, KH, P], bf16)
            wu_sb = persist.tile([P, NI, KH, P], bf16)

            def w_src(w, it):
                return w.ap()[it].rearrange("p (k i) -> p k i", i=P)

            def load_w(it):
                nc.sync.dma_start(wg_sb[:, it], w_src(wg, it))
                nc.scalar.dma_start(wu_sb[:, it], w_src(wu, it))

            # Startup: first weight tiles and the first-sweep xt halves, fine
            # grained and split across both HWDGE rings so the first matmuls
            # unblock on small transfers.
            nc.sync.dma_start(wg_sb[:, 0], w_src(wg, 0))
            nc.scalar.dma_start(xt_sb[:, 0, 0:c0], xt.ap()[0][:, 0:c0])
            nc.scalar.dma_start(wu_sb[:, 0], w_src(wu, 0))
            for k in range(1, KH):
                eng = nc.sync if k % 2 == 1 else nc.scalar
                eng.dma_start(xt_sb[:, k, 0:c0], xt.ap()[k][:, 0:c0])
            load_w(1)

            gelu = mybir.ActivationFunctionType.Gelu_apprx_tanh

            # Phase 1: per m-chunk sweep over i-tiles:
            #   gT/uT = wg^T xT / wu^T xT ; act = gelu(g) * u
            for j, (m0, ln) in enumerate(chunks):
                for it in range(NI):
                    if j == 0:
                        # stream remaining weights / second-sweep xt / wd
                        pf = it + 2
                        if 2 <= pf < NI:
                            load_w(pf)
                        if 0 <= it < 4:
                            # second-sweep xt slices (k pairs per iteration)
                            nc.sync.dma_start(
                                xt_sb[:, 2 * it, c0:], xt.ap()[2 * it][:, c0:]
                            )
                            nc.scalar.dma_start(
                                xt_sb[:, 2 * it + 1, c0:],
                                xt.ap()[2 * it + 1][:, c0:],
                            )
                        if 4 <= it < 12:
                            w0 = 2 * (it - 4)
                            nc.sync.dma_start(
                                wd_sb[:, w0], wd.ap()[w0 * P:(w0 + 1) * P, :]
                            )
                            nc.scalar.dma_start(
                                wd_sb[:, w0 + 1],
                                wd.ap()[(w0 + 1) * P:(w0 + 2) * P, :],
                            )

                    g_ps = psA.tile([P, ln], f32, tag="g", name=f"g_{j}_{it}")
                    for k in range(KH):
                        nc.tensor.matmul(
                            g_ps[:],
                            wg_sb[:, it, k],
                            xt_sb[:, k, m0:m0 + ln],
                            start=(k == 0),
                            stop=(k == KH - 1),
                        )
                    u_ps = psA.tile([P, ln], f32, tag="u", name=f"u_{j}_{it}")
                    for k in range(KH):
                        nc.tensor.matmul(
                            u_ps[:],
                            wu_sb[:, it, k],
                            xt_sb[:, k, m0:m0 + ln],
                            start=(k == 0),
                            stop=(k == KH - 1),
                        )
                    nc.scalar.activation(acts[:, it, m0:m0 + ln], g_ps[:], gelu)
                    nc.vector.tensor_mul(
                        acts[:, it, m0:m0 + ln], acts[:, it, m0:m0 + ln], u_ps[:]
                    )

            # Phase 2: yT[h, m] = sum_i wd[i, h] * act[i, m].  The last h-tile
            # uses quarter-size m chunks so the final copy+DMA tail is short.
            for ht in range(HT):
                mparts = chunks if ht < HT - 1 else quarters
                d_ps = [
                    psB.tile([P, ln], f32, tag=f"d{jj % 2}", name=f"d{jj}_{ht}")
                    for jj, (m0, ln) in enumerate(mparts)
                ]
                for it in range(NI):
                    wslice = wd_sb[:, it, ht * P:(ht + 1) * P]
                    for jj, (m0, ln) in enumerate(mparts):
                        nc.tensor.matmul(
                            d_ps[jj][:],
                            wslice,
                            acts[:, it, m0:m0 + ln],
                            start=(it == 0),
                            stop=(it == NI - 1),
                        )
                for jj, (m0, ln) in enumerate(mparts):
                    y_sb = youts.tile(
                        [P, ln], bf16, tag=f"y{jj % 2}", name=f"y{jj}_{ht}"
                    )
                    if jj % 2 == 0:
                        nc.scalar.copy(y_sb[:], d_ps[jj][:])
                        nc.sync.dma_start(y.ap()[ht, :, m0:m0 + ln], y_sb[:])
                    else:
                        nc.vector.tensor_copy(y_sb[:], d_ps[jj][:])
                        nc.scalar.dma_start(y.ap()[ht, :, m0:m0 + ln], y_sb[:])

    nc.compile()
    return nc


def _pack_w(w_e):
    """[H, I] -> [NI, P, KH*P] bf16 with w[it, p, k*P+i] = w_e[k*P+p, it*P+i]."""
    w4 = w_e.reshape(KH, P, NI, P)
    return np.ascontiguousarray(
        w4.transpose(2, 1, 0, 3).reshape(NI, P, KH * P).astype(BF16)
    )


def kernel(hidden_states, gate_w, wg, wu, wd):
    global LAST_RESULTS
    x = np.ascontiguousarray(np.asarray(hidden_states, dtype=np.float32))
    gw = np.asarray(gate_w, dtype=np.float32)
    wg = np.asarray(wg, dtype=np.float32)
    wu = np.asarray(wu, dtype=np.float32)
    wd = np.asarray(wd, dtype=np.float32)

    # Router on host (part of the dispatch/sharding step).
    logits = np.tanh((x @ gw) / np.float32(SOFTCAP))
    top2 = np.argsort(-logits, axis=1, kind="stable")[:, :TOPK]  # [T, 2]
    v = np.take_along_axis(logits, top2, axis=1)                 # descending
    ex = np.exp(v - v[:, :1])
    pk = (ex / ex.sum(axis=1, keepdims=True)).astype(np.float32)  # [T, 2]

    token_ids, probs_e = [], []
    for e in range(E):
        mask = top2 == e
        rows = np.where(mask.any(axis=1))[0]
        kk = np.argmax(mask[rows], axis=1)
        token_ids.append(rows)
        probs_e.append(pk[rows, kk])

    n_max = max(len(r) for r in token_ids)
    M_PAD = max(64, -(-n_max // 16) * 16)  # both m-chunks equal multiples of 8

    nc = _compiled.get(M_PAD)
    if nc is None:
        nc = _build(M_PAD)
        _compiled[M_PAD] = nc

    in_maps = []
    for e in range(E):
        ids = token_ids[e]
        xe = np.zeros((M_PAD, H), np.float32)
        xe[: len(ids)] = x[ids]
        # [M_PAD, KH, P] -> [KH, P, M_PAD]
        xt_e = np.ascontiguousarray(
            xe.reshape(M_PAD, KH, P).transpose(1, 2, 0).astype(BF16)
        )
        in_maps.append(
            {
                "xt": xt_e,
                "wg": _pack_w(wg[e]),
                "wu": _pack_w(wu[e]),
                "wd": np.ascontiguousarray(wd[e].astype(BF16)),
            }
        )

    res = run_bass_kernel_spmd(nc, in_maps, core_ids=list(range(N_CORES)))
    LAST_RESULTS = res

    out = np.zeros((T, H), np.float32)
    for e in range(E):
        ids = token_ids[e]
        y_e = res.results[e]["y"].reshape(H, M_PAD).astype(np.float32)
        out[ids] += probs_e[e][:, None] * y_e[:, : len(ids)].T
    return out


# revision 4
# speedup vs baseline: 1.0392x; 1.0392x over previous
"""Grok1-style MoE (T=2048, H=1024, E=8, I=2048, top-2) on 8 Trainium2 cores.

Strategy (expert-parallel, per the sharding hint):
  - Host: compute the tiny router (x @ gate_w, tanh softcap, top-2, softmax)
    and dispatch tokens by expert assignment (the "all-to-all dispatch" step:
    with full inputs on the host, dispatch = gather per expert), packing the
    per-core shards in the device-friendly tiled layout.
  - Device (SPMD, 1 expert per core), all-bf16 matmuls (enables the PE's
    fast-weight-load path and halves HBM traffic; fp32 PSUM accumulate):
      Phase 1:  gT/uT[i, m] = wg_e^T x_e^T / wu_e^T x_e^T  per 128-i-tile
                act[i, m]   = gelu_tanh(gT) * uT            (stored bf16)
      Phase 2:  yT[h, m]    = sum_i wd_e[i, h] * act[i, m]  (exact-M moving
                dim -- no 128-token quantization of the output pass); the
        last h-tile completes its two m-chunks one after the other so the
        final output copy+DMA drains staggered instead of all at the end.
  - Host: combine = out[ids] += prob * yT.T per expert (prob scaling is
    linear in the expert output, so it folds into the host-side combine).
"""

import numpy as np
import ml_dtypes

import concourse.mybir as mybir
import concourse.tile as tile
from concourse import bacc
from concourse.bass_utils import run_bass_kernel_spmd

T, H, E, I_DIM, TOPK = 2048, 1024, 8, 2048, 2
SOFTCAP = 30.0
P = 128
N_CORES = 8
KH = H // P      # 8 contraction tiles (phase 1)
NI = I_DIM // P  # 16 i tiles
HT = H // P      # 8 h tiles (phase 2 output partitions)

BF16 = ml_dtypes.bfloat16

_compiled = {}
LAST_RESULTS = None


def _build(M_PAD):
    c0 = M_PAD // 2
    chunks = ((0, c0), (c0, M_PAD - c0))
    f32 = mybir.dt.float32
    bf16 = mybir.dt.bfloat16

    nc = bacc.Bacc("TRN2", target_bir_lowering=False, num_devices=N_CORES)
    # Host-packed layouts (all DMAs contiguous per partition):
    #   xt  [KH, P, M_PAD]   : xt[k, p, m] = x_e[m, k*P+p]
    #   wg  [NI, P, KH*P]    : wg[it, p, k*P+i] = wg_e[k*P+p, it*P+i]
    #   wu  same as wg
    #   wd  [I, H]           : natural layout (row-tile slices are contiguous)
    #   y   [HT, P, M_PAD]   : y[ht, p, m] = yT_e[ht*P+p, m]
    xt = nc.dram_tensor("xt", [KH, P, M_PAD], bf16, kind="ExternalInput")
    wg = nc.dram_tensor("wg", [NI, P, KH * P], bf16, kind="ExternalInput")
    wu = nc.dram_tensor("wu", [NI, P, KH * P], bf16, kind="ExternalInput")
    wd = nc.dram_tensor("wd", [I_DIM, H], bf16, kind="ExternalInput")
    y = nc.dram_tensor("y", [HT, P, M_PAD], bf16, kind="ExternalOutput")

    with tile.TileContext(nc) as tc:
        with (
            tc.tile_pool(name="persist", bufs=1) as persist,
            tc.tile_pool(name="youts", bufs=2) as youts,
            tc.tile_pool(name="psG", bufs=2, space="PSUM") as psG,
            tc.tile_pool(name="psU", bufs=1, space="PSUM") as psU,
            tc.tile_pool(name="psD", bufs=1, space="PSUM") as psD,
        ):
            xt_sb = persist.tile([P, KH, M_PAD], bf16)
            wd_sb = persist.tile([P, NI, H], bf16)
            acts = persist.tile([P, NI, M_PAD], bf16)
            wg_sb = persist.tile([P, NI, KH, P], bf16)
            wu_sb = persist.tile([P, NI, KH, P], bf16)

            def w_src(w, it):
                return w.ap()[it].rearrange("p (k i) -> p k i", i=P)

            def load_w(it):
                nc.sync.dma_start(wg_sb[:, it], w_src(wg, it))
                nc.scalar.dma_start(wu_sb[:, it], w_src(wu, it))

            # Startup: first weight tile fine-grained (the first matmul only
            # needs wg0[k=0]), xt k-slices split across both HWDGE rings.
            nc.sync.dma_start(wg_sb[:, 0, 0:1], w_src(wg, 0)[:, 0:1])
            nc.scalar.dma_start(xt_sb[:, 0], xt.ap()[0])
            nc.sync.dma_start(wg_sb[:, 0, 1:], w_src(wg, 0)[:, 1:])
            nc.scalar.dma_start(wu_sb[:, 0, 0:2], w_src(wu, 0)[:, 0:2])
            for k in range(1, KH):
                eng = nc.sync if k % 2 == 1 else nc.scalar
                eng.dma_start(xt_sb[:, k], xt.ap()[k])
            nc.scalar.dma_start(wu_sb[:, 0, 2:], w_src(wu, 0)[:, 2:])
            load_w(1)

            gelu = mybir.ActivationFunctionType.Gelu_apprx_tanh

            # Phase 1: per i-tile: gT/uT = wg^T xT / wu^T xT; act = gelu(g)*u.
            # Both m-chunks per k so each stationary tile serves 2 matmuls.
            for it in range(NI):
                pf = it + 2
                if 2 <= pf < NI:
                    load_w(pf)
                # wd tile loads are spread over the phase-1 steady state
                # (consumed only in phase 2).
                if 2 <= it <= 9:
                    w0 = 2 * (it - 2)
                    nc.sync.dma_start(
                        wd_sb[:, w0], wd.ap()[w0 * P:(w0 + 1) * P, :]
                    )
                    nc.scalar.dma_start(
                        wd_sb[:, w0 + 1], wd.ap()[(w0 + 1) * P:(w0 + 2) * P, :]
                    )

                g_ps = [
                    psG.tile([P, ln], f32, tag=f"g{j}", name=f"g{j}_{it}")
                    for j, (m0, ln) in enumerate(chunks)
                ]
                for k in range(KH):
                    for j, (m0, ln) in enumerate(chunks):
                        nc.tensor.matmul(
                            g_ps[j][:],
                            wg_sb[:, it, k],
                            xt_sb[:, k, m0:m0 + ln],
                            start=(k == 0),
                            stop=(k == KH - 1),
                        )
                u_ps = [
                    psU.tile([P, ln], f32, tag=f"u{j}", name=f"u{j}_{it}")
                    for j, (m0, ln) in enumerate(chunks)
                ]
                for k in range(KH):
                    for j, (m0, ln) in enumerate(chunks):
                        nc.tensor.matmul(
                            u_ps[j][:],
                            wu_sb[:, it, k],
                            xt_sb[:, k, m0:m0 + ln],
                            start=(k == 0),
                            stop=(k == KH - 1),
                        )
                for j, (m0, ln) in enumerate(chunks):
                    nc.scalar.activation(acts[:, it, m0:m0 + ln], g_ps[j][:], gelu)
                    nc.vector.tensor_mul(
                        acts[:, it, m0:m0 + ln], acts[:, it, m0:m0 + ln], u_ps[j][:]
                    )

            # Phase 2: yT[h, m] = sum_i wd[i, h] * act[i, m].  For the last
            # h-tile, the two m-chunks run sequentially so the final output
            # copies/DMAs stagger instead of stacking after the last matmul.
            def d_group(ht, parts):
                d_ps = [
                    psD.tile([P, ln], f32, tag=f"d{jj}", name=f"d{jj}_{ht}")
                    for jj, (m0, ln) in enumerate(parts)
                ]
                for it in range(NI):
                    wslice = wd_sb[:, it, ht * P:(ht + 1) * P]
                    for jj, (m0, ln) in enumerate(parts):
                        nc.tensor.matmul(
                            d_ps[jj][:],
                            wslice,
                            acts[:, it, m0:m0 + ln],
                            start=(it == 0),
                            stop=(it == NI - 1),
                        )
                return d_ps

            def d_store(ht, jj, m0, ln, d_ps):
                y_sb = youts.tile([P, ln], bf16, tag=f"y{jj}", name=f"y{jj}_{ht}")
                if jj == 0:
                    nc.scalar.copy(y_sb[:], d_ps[:])
                    nc.sync.dma_start(y.ap()[ht, :, m0:m0 + ln], y_sb[:])
                else:
                    nc.vector.tensor_copy(y_sb[:], d_ps[:])
                    nc.scalar.dma_start(y.ap()[ht, :, m0:m0 + ln], y_sb[:])

            for ht in range(HT - 1):
                d_ps = d_group(ht, chunks)
                for jj, (m0, ln) in enumerate(chunks):
                    d_store(ht, jj, m0, ln, d_ps[jj])
            ht = HT - 1
            for jj, (m0, ln) in enumerate(chunks):
                (d_ps,) = d_group(ht, (chunks[jj],))
                d_store(ht, jj, m0, ln, d_ps)

    nc.compile()
    return nc


def _pack_w(w_e):
    """[H, I] -> [NI, P, KH*P] bf16 with w[it, p, k*P+i] = w_e[k*P+p, it*P+i]."""
    w4 = w_e.reshape(KH, P, NI, P)
    return np.ascontiguousarray(
        w4.transpose(2, 1, 0, 3).reshape(NI, P, KH * P).astype(BF16)
    )


def kernel(hidden_states, gate_w, wg, wu, wd):
    global LAST_RESULTS
    x = np.ascontiguousarray(np.asarray(hidden_states, dtype=np.float32))
    gw = np.asarray(gate_w, dtype=np.float32)
    wg = np.asarray(wg, dtype=np.float32)
    wu = np.asarray(wu, dtype=np.float32)
    wd = np.asarray(wd, dtype=np.float32)

    # Router on host (part of the dispatch/sharding step).
    logits = np.tanh((x @ gw) / np.float32(SOFTCAP))
    top2 = np.argsort(-logits, axis=1, kind="stable")[:, :TOPK]  # [T, 2]
    v = np.take_along_axis(logits, top2, axis=1)                 # descending
    ex = np.exp(v - v[:, :1])
    pk = (ex / ex.sum(axis=1, keepdims=True)).astype(np.float32)  # [T, 2]

    token_ids, probs_e = [], []
    for e in range(E):
        mask = top2 == e
        rows = np.where(mask.any(axis=1))[0]
        kk = np.argmax(mask[rows], axis=1)
        token_ids.append(rows)
        probs_e.append(pk[rows, kk])

    n_max = max(len(r) for r in token_ids)
    M_PAD = max(64, -(-n_max // 16) * 16)  # both m-chunks equal multiples of 8

    nc = _compiled.get(M_PAD)
    if nc is None:
        nc = _build(M_PAD)
        _compiled[M_PAD] = nc

    in_maps = []
    for e in range(E):
        ids = token_ids[e]
        xe = np.zeros((M_PAD, H), np.float32)
        xe[: len(ids)] = x[ids]
        # [M_PAD, KH, P] -> [KH, P, M_PAD]
        xt_e = np.ascontiguousarray(
            xe.reshape(M_PAD, KH, P).transpose(1, 2, 0).astype(BF16)
        )
        in_maps.append(
            {
                "xt": xt_e,
                "wg": _pack_w(wg[e]),
                "wu": _pack_w(wu[e]),
                "wd": np.ascontiguousarray(wd[e].astype(BF16)),
            }
        )

    res = run_bass_kernel_spmd(nc, in_maps, core_ids=list(range(N_CORES)))
    LAST_RESULTS = res

    out = np.zeros((T, H), np.float32)
    for e in range(E):
        ids = token_ids[e]
        y_e = res.results[e]["y"].reshape(H, M_PAD).astype(np.float32)
        out[ids] += probs_e[e][:, None] * y_e[:, : len(ids)].T
    return out


# revision 5
# speedup vs baseline: 1.0440x; 1.0046x over previous
"""Grok1-style MoE (T=2048, H=1024, E=8, I=2048, top-2) on 8 Trainium2 cores.

Strategy (expert-parallel, per the sharding hint):
  - Host: compute the tiny router (x @ gate_w, tanh softcap, top-2, softmax)
    and dispatch tokens by expert assignment (the "all-to-all dispatch" step:
    with full inputs on the host, dispatch = gather per expert), packing the
    per-core shards in the device-friendly tiled layout.
  - Device (SPMD, 1 expert per core), all-bf16 matmuls (enables the PE's
    fast-weight-load path and halves HBM traffic; fp32 PSUM accumulate):
      Phase 1:  gT/uT[i, m] = wg_e^T x_e^T / wu_e^T x_e^T  per 128-i-tile
                act[i, m]   = gelu_tanh(gT) * uT            (stored bf16)
      Phase 2:  yT[h, m]    = sum_i wd_e[i, h] * act[i, m]  (exact-M moving
                dim -- no 128-token quantization of the output pass); the
        last h-tile completes its two m-chunks one after the other so the
        final output copy+DMA drains staggered instead of all at the end.
  - Host: combine = out[ids] += prob * yT.T per expert (prob scaling is
    linear in the expert output, so it folds into the host-side combine).
"""

import numpy as np
import ml_dtypes

import concourse.mybir as mybir
import concourse.tile as tile
from concourse import bacc
from concourse.bass_utils import run_bass_kernel_spmd

T, H, E, I_DIM, TOPK = 2048, 1024, 8, 2048, 2
SOFTCAP = 30.0
P = 128
N_CORES = 8
KH = H // P      # 8 contraction tiles (phase 1)
NI = I_DIM // P  # 16 i tiles
HT = H // P      # 8 h tiles (phase 2 output partitions)

BF16 = ml_dtypes.bfloat16

_compiled = {}
LAST_RESULTS = None


def _build(M_PAD):
    c0 = M_PAD // 2
    chunks = ((0, c0), (c0, M_PAD - c0))
    f32 = mybir.dt.float32
    bf16 = mybir.dt.bfloat16

    nc = bacc.Bacc("TRN2", target_bir_lowering=False, num_devices=N_CORES)
    # Host-packed layouts (all DMAs contiguous per partition):
    #   xt  [KH, P, M_PAD]   : xt[k, p, m] = x_e[m, k*P+p]
    #   wg  [NI, P, KH*P]    : wg[it, p, k*P+i] = wg_e[k*P+p, it*P+i]
    #   wu  same as wg
    #   wd  [I, H]           : natural layout (row-tile slices are contiguous)
    #   y   [HT, P, M_PAD]   : y[ht, p, m] = yT_e[ht*P+p, m]
    xt = nc.dram_tensor("xt", [KH, P, M_PAD], bf16, kind="ExternalInput")
    wg = nc.dram_tensor("wg", [NI, P, KH * P], bf16, kind="ExternalInput")
    wu = nc.dram_tensor("wu", [NI, P, KH * P], bf16, kind="ExternalInput")
    wd = nc.dram_tensor("wd", [I_DIM, H], bf16, kind="ExternalInput")
    y = nc.dram_tensor("y", [HT, P, M_PAD], bf16, kind="ExternalOutput")

    with tile.TileContext(nc) as tc:
        with (
            tc.tile_pool(name="persist", bufs=1) as persist,
            tc.tile_pool(name="youts", bufs=2) as youts,
            tc.tile_pool(name="psG", bufs=1, space="PSUM") as psG,
            tc.tile_pool(name="psU", bufs=1, space="PSUM") as psU,
            tc.tile_pool(name="psD", bufs=2, space="PSUM") as psD,
        ):
            xt_sb = persist.tile([P, KH, M_PAD], bf16)
            wd_sb = persist.tile([P, NI, H], bf16)
            acts = persist.tile([P, NI, M_PAD], bf16)
            wg_sb = persist.tile([P, NI, KH, P], bf16)
            wu_sb = persist.tile([P, NI, KH, P], bf16)

            def w_src(w, it):
                return w.ap()[it].rearrange("p (k i) -> p k i", i=P)

            def load_w(it):
                nc.sync.dma_start(wg_sb[:, it], w_src(wg, it))
                nc.scalar.dma_start(wu_sb[:, it], w_src(wu, it))

            # Startup: first weight tile fine-grained (the first matmul only
            # needs wg0[k=0] and the first m-chunk of xt[k=0]), xt k-slices
            # split across both HWDGE rings.
            nc.sync.dma_start(wg_sb[:, 0, 0:1], w_src(wg, 0)[:, 0:1])
            nc.scalar.dma_start(xt_sb[:, 0, 0:c0], xt.ap()[0][:, 0:c0])
            nc.scalar.dma_start(xt_sb[:, 0, c0:], xt.ap()[0][:, c0:])
            nc.sync.dma_start(wg_sb[:, 0, 1:], w_src(wg, 0)[:, 1:])
            nc.scalar.dma_start(wu_sb[:, 0], w_src(wu, 0))
            for k in range(1, KH):
                eng = nc.sync if k % 2 == 1 else nc.scalar
                eng.dma_start(xt_sb[:, k], xt.ap()[k])
            load_w(1)

            gelu = mybir.ActivationFunctionType.Gelu_apprx_tanh

            # Phase 1: per i-tile: gT/uT = wg^T xT / wu^T xT; act = gelu(g)*u.
            # Both m-chunks per k so each stationary tile serves 2 matmuls.
            wd_next = 0

            def load_wd():
                nonlocal wd_next
                i0 = wd_next * P
                eng = nc.sync if wd_next % 2 == 0 else nc.scalar
                eng.dma_start(wd_sb[:, wd_next], wd.ap()[i0:i0 + P, :])
                wd_next += 1

            for it in range(NI):
                g_ps = [
                    psG.tile([P, ln], f32, tag=f"g{j}", name=f"g{j}_{it}")
                    for j, (m0, ln) in enumerate(chunks)
                ]
                for k in range(KH):
                    for j, (m0, ln) in enumerate(chunks):
                        nc.tensor.matmul(
                            g_ps[j][:],
                            wg_sb[:, it, k],
                            xt_sb[:, k, m0:m0 + ln],
                            start=(k == 0),
                            stop=(k == KH - 1),
                        )
                u_ps = [
                    psU.tile([P, ln], f32, tag=f"u{j}", name=f"u{j}_{it}")
                    for j, (m0, ln) in enumerate(chunks)
                ]
                for k in range(KH):
                    for j, (m0, ln) in enumerate(chunks):
                        nc.tensor.matmul(
                            u_ps[j][:],
                            wu_sb[:, it, k],
                            xt_sb[:, k, m0:m0 + ln],
                            start=(k == 0),
                            stop=(k == KH - 1),
                        )
                for j, (m0, ln) in enumerate(chunks):
                    nc.scalar.activation(acts[:, it, m0:m0 + ln], g_ps[j][:], gelu)
                    nc.vector.tensor_mul(
                        acts[:, it, m0:m0 + ln], acts[:, it, m0:m0 + ln], u_ps[j][:]
                    )
                # DMA pushes go after the compute ops so the scalar queue's
                # activations are not stuck behind descriptor pushes.
                pf = it + 2
                if 2 <= pf < NI:
                    load_w(pf)
                # wd tile loads are spread over the phase-1 steady state
                # (consumed only in phase 2).
                if it >= 2:
                    load_wd()
                    if it >= NI - 2:
                        load_wd()

            # Phase 2: yT[h, m] = sum_i wd[i, h] * act[i, m].  For the last
            # h-tile, the two m-chunks run sequentially so the final output
            # copies/DMAs stagger instead of stacking after the last matmul.
            def d_group(ht, parts):
                d_ps = [
                    psD.tile([P, ln], f32, tag=f"d{jj}", name=f"d{jj}_{ht}")
                    for jj, (m0, ln) in enumerate(parts)
                ]
                for it in range(NI):
                    wslice = wd_sb[:, it, ht * P:(ht + 1) * P]
                    for jj, (m0, ln) in enumerate(parts):
                        nc.tensor.matmul(
                            d_ps[jj][:],
                            wslice,
                            acts[:, it, m0:m0 + ln],
                            start=(it == 0),
                            stop=(it == NI - 1),
                        )
                return d_ps

            def d_store(ht, jj, m0, ln, d_ps):
                y_sb = youts.tile([P, ln], bf16, tag=f"y{jj}", name=f"y{jj}_{ht}")
                if jj == 0:
                    nc.scalar.copy(y_sb[:], d_ps[:])
                    nc.sync.dma_start(y.ap()[ht, :, m0:m0 + ln], y_sb[:])
                else:
                    nc.vector.tensor_copy(y_sb[:], d_ps[:])
                    nc.scalar.dma_start(y.ap()[ht, :, m0:m0 + ln], y_sb[:])

            c3 = 3 * (M_PAD // 4)
            last_chunks = ((0, c3), (c3, M_PAD - c3))
            for ht in range(HT):
                parts = chunks if ht < HT - 1 else last_chunks
                d_ps = d_group(ht, parts)
                for jj, (m0, ln) in enumerate(parts):
                    d_store(ht, jj, m0, ln, d_ps[jj])

    nc.compile()
    return nc


def _pack_w(w_e):
    """[H, I] -> [NI, P, KH*P] bf16 with w[it, p, k*P+i] = w_e[k*P+p, it*P+i]."""
    w4 = w_e.reshape(KH, P, NI, P)
    return np.ascontiguousarray(
        w4.transpose(2, 1, 0, 3).reshape(NI, P, KH * P).astype(BF16)
    )


def kernel(hidden_states, gate_w, wg, wu, wd):
    global LAST_RESULTS
    x = np.ascontiguousarray(np.asarray(hidden_states, dtype=np.float32))
    gw = np.asarray(gate_w, dtype=np.float32)
    wg = np.asarray(wg, dtype=np.float32)
    wu = np.asarray(wu, dtype=np.float32)
    wd = np.asarray(wd, dtype=np.float32)

    # Router on host (part of the dispatch/sharding step).
    logits = np.tanh((x @ gw) / np.float32(SOFTCAP))
    top2 = np.argsort(-logits, axis=1, kind="stable")[:, :TOPK]  # [T, 2]
    v = np.take_along_axis(logits, top2, axis=1)                 # descending
    ex = np.exp(v - v[:, :1])
    pk = (ex / ex.sum(axis=1, keepdims=True)).astype(np.float32)  # [T, 2]

    token_ids, probs_e = [], []
    for e in range(E):
        mask = top2 == e
        rows = np.where(mask.any(axis=1))[0]
        kk = np.argmax(mask[rows], axis=1)
        token_ids.append(rows)
        probs_e.append(pk[rows, kk])

    n_max = max(len(r) for r in token_ids)
    M_PAD = max(64, -(-n_max // 16) * 16)  # both m-chunks equal multiples of 8

    nc = _compiled.get(M_PAD)
    if nc is None:
        nc = _build(M_PAD)
        _compiled[M_PAD] = nc

    in_maps = []
    for e in range(E):
        ids = token_ids[e]
        xe = np.zeros((M_PAD, H), np.float32)
        xe[: len(ids)] = x[ids]
        # [M_PAD, KH, P] -> [KH, P, M_PAD]
        xt_e = np.ascontiguousarray(
            xe.reshape(M_PAD, KH, P).transpose(1, 2, 0).astype(BF16)
        )
        in_maps.append(
            {
                "xt": xt_e,
                "wg": _pack_w(wg[e]),
                "wu": _pack_w(wu[e]),
                "wd": np.ascontiguousarray(wd[e].astype(BF16)),
            }
        )

    res = run_bass_kernel_spmd(nc, in_maps, core_ids=list(range(N_CORES)))
    LAST_RESULTS = res

    out = np.zeros((T, H), np.float32)
    for e in range(E):
        ids = token_ids[e]
        y_e = res.results[e]["y"].reshape(H, M_PAD).astype(np.float32)
        out[ids] += probs_e[e][:, None] * y_e[:, : len(ids)].T
    return out


# revision 7
# speedup vs baseline: 1.0445x; 1.0004x over previous
"""Grok1-style MoE (T=2048, H=1024, E=8, I=2048, top-2) on 8 Trainium2 cores.

Strategy (expert-parallel, per the sharding hint):
  - Host: compute the tiny router (x @ gate_w, tanh softcap, top-2, softmax)
    and dispatch tokens by expert assignment (the "all-to-all dispatch" step:
    with full inputs on the host, dispatch = gather per expert), packing the
    per-core shards in the device-friendly tiled layout.
  - Device (SPMD, 1 expert per core), all-bf16 matmuls (enables the PE's
    fast-weight-load path and halves HBM traffic; fp32 PSUM accumulate):
      Phase 1:  gT/uT[i, m] = wg_e^T x_e^T / wu_e^T x_e^T  per 128-i-tile
                act[i, m]   = gelu_tanh(gT) * uT            (stored bf16)
      Phase 2:  yT[h, m]    = sum_i wd_e[i, h] * act[i, m]  (exact-M moving
                dim -- no 128-token quantization of the output pass)
  - Host: combine = out[ids] += prob * yT.T per expert (prob scaling is
    linear in the expert output, so it folds into the host-side combine).

Scheduling notes (from trace analysis):
  - Weight/wd tiles stream as paired 512KB DMAs: each HWDGE ring has ~4
    completion lanes with ~1.4us fixed latency per DMA, so 256KB singles
    sustain only ~100GB/s/ring -- below the weight stream demand.
  - Throwaway PE warmup matmuls run during the startup DMA window so the
    p-state ramp is spent before real work begins.
"""

import numpy as np
import ml_dtypes

import concourse.mybir as mybir
import concourse.tile as tile
from concourse import bacc
from concourse.bass_utils import run_bass_kernel_spmd

T, H, E, I_DIM, TOPK = 2048, 1024, 8, 2048, 2
SOFTCAP = 30.0
P = 128
N_CORES = 8
KH = H // P      # 8 contraction tiles (phase 1)
NI = I_DIM // P  # 16 i tiles
HT = H // P      # 8 h tiles (phase 2 output partitions)

BF16 = ml_dtypes.bfloat16

_compiled = {}
LAST_RESULTS = None


def _build(M_PAD):
    c0 = M_PAD // 2
    chunks = ((0, c0), (c0, M_PAD - c0))
    f32 = mybir.dt.float32
    bf16 = mybir.dt.bfloat16

    nc = bacc.Bacc("TRN2", target_bir_lowering=False, num_devices=N_CORES)
    # Host-packed layouts (all DMAs contiguous per partition):
    #   xt  [KH, P, M_PAD]   : xt[k, p, m] = x_e[m, k*P+p]
    #   wg  [NI, P, KH*P]    : wg[it, p, k*P+i] = wg_e[k*P+p, it*P+i]
    #   wu  same as wg
    #   wd  [I, H]           : natural layout (row-tile slices are contiguous)
    #   y   [HT, P, M_PAD]   : y[ht, p, m] = yT_e[ht*P+p, m]
    xt = nc.dram_tensor("xt", [KH, P, M_PAD], bf16, kind="ExternalInput")
    wg = nc.dram_tensor("wg", [NI, P, KH * P], bf16, kind="ExternalInput")
    wu = nc.dram_tensor("wu", [NI, P, KH * P], bf16, kind="ExternalInput")
    wd = nc.dram_tensor("wd", [I_DIM, H], bf16, kind="ExternalInput")
    y = nc.dram_tensor("y", [HT, P, M_PAD], bf16, kind="ExternalOutput")

    with tile.TileContext(nc) as tc:
        with (
            tc.tile_pool(name="persist", bufs=1) as persist,
            tc.tile_pool(name="youts", bufs=2) as youts,
            tc.tile_pool(name="psG", bufs=1, space="PSUM") as psG,
            tc.tile_pool(name="psU", bufs=1, space="PSUM") as psU,
            tc.tile_pool(name="psD", bufs=2, space="PSUM") as psD,
        ):
            xt_sb = persist.tile([P, KH, M_PAD], bf16)
            wd_sb = persist.tile([P, NI, H], bf16)
            acts = persist.tile([P, NI, M_PAD], bf16)
            wg_sb = persist.tile([P, NI, KH, P], bf16)
            wu_sb = persist.tile([P, NI, KH, P], bf16)

            def w_src(w, it):
                return w.ap()[it].rearrange("p (k i) -> p k i", i=P)

            def w_src2(w, it):
                return w.ap()[it:it + 2].rearrange("t p (k i) -> p t k i", i=P)

            def load_w(it):
                nc.sync.dma_start(wg_sb[:, it], w_src(wg, it))
                nc.scalar.dma_start(wu_sb[:, it], w_src(wu, it))

            def load_w2(it):
                nc.sync.dma_start(wg_sb[:, it:it + 2], w_src2(wg, it))
                nc.scalar.dma_start(wu_sb[:, it:it + 2], w_src2(wu, it))

            # Startup: first weight tile fine-grained (the first matmul only
            # needs wg0[k=0] and the first m-chunk of xt[k=0]), xt k-slices
            # split across both HWDGE rings.
            nc.sync.dma_start(wg_sb[:, 0, 0:1], w_src(wg, 0)[:, 0:1])
            nc.scalar.dma_start(xt_sb[:, 0, 0:c0], xt.ap()[0][:, 0:c0])
            nc.scalar.dma_start(xt_sb[:, 0, c0:], xt.ap()[0][:, c0:])
            nc.sync.dma_start(wg_sb[:, 0, 1:], w_src(wg, 0)[:, 1:])
            nc.scalar.dma_start(wu_sb[:, 0], w_src(wu, 0))
            for k in range(1, KH):
                eng = nc.sync if k % 2 == 1 else nc.scalar
                eng.dma_start(xt_sb[:, k], xt.ap()[k])
            load_w2(1)

            gelu = mybir.ActivationFunctionType.Gelu_apprx_tanh

            # PE warmup: throwaway matmuls on the first-arrived tiles keep
            # the PE busy (and ramping to full clock) while the bulk of xt
            # and the early weight tiles stream in.
            warm = psG.tile([P, c0], f32, tag="g0", name="warm")
            for _ in range(16):
                nc.tensor.matmul(
                    warm[:], wg_sb[:, 0, 0], xt_sb[:, 0, 0:c0],
                    start=True, stop=True,
                )

            wd_next = 0

            def load_wd2():
                nonlocal wd_next
                i0 = wd_next * P
                eng = nc.sync if wd_next % 4 == 0 else nc.scalar
                eng.dma_start(
                    wd_sb[:, wd_next:wd_next + 2],
                    wd.ap()[i0:i0 + 2 * P, :].rearrange("(t p) h -> p t h", p=P),
                )
                wd_next += 2

            # Phase 1: per i-tile: gT/uT = wg^T xT / wu^T xT; act = gelu(g)*u.
            # Both m-chunks per k so each stationary tile serves 2 matmuls.
            for it in range(NI):
                if it % 2 == 1 and it + 2 < NI:
                    if it + 3 < NI:
                        load_w2(it + 2)
                    else:
                        load_w(it + 2)
                if 2 <= it <= 9:
                    load_wd2()

                g_ps = [
                    psG.tile([P, ln], f32, tag=f"g{j}", name=f"g{j}_{it}")
                    for j, (m0, ln) in enumerate(chunks)
                ]
                for k in range(KH):
                    for j, (m0, ln) in enumerate(chunks):
                        nc.tensor.matmul(
                            g_ps[j][:],
                            wg_sb[:, it, k],
                            xt_sb[:, k, m0:m0 + ln],
                            start=(k == 0),
                            stop=(k == KH - 1),
                        )
                u_ps = [
                    psU.tile([P, ln], f32, tag=f"u{j}", name=f"u{j}_{it}")
                    for j, (m0, ln) in enumerate(chunks)
                ]
                for k in range(KH):
                    for j, (m0, ln) in enumerate(chunks):
                        nc.tensor.matmul(
                            u_ps[j][:],
                            wu_sb[:, it, k],
                            xt_sb[:, k, m0:m0 + ln],
                            start=(k == 0),
                            stop=(k == KH - 1),
                        )
                for j, (m0, ln) in enumerate(chunks):
                    nc.scalar.activation(acts[:, it, m0:m0 + ln], g_ps[j][:], gelu)
                    nc.vector.tensor_mul(
                        acts[:, it, m0:m0 + ln], acts[:, it, m0:m0 + ln], u_ps[j][:]
                    )

            # Phase 2: yT[h, m] = sum_i wd[i, h] * act[i, m].  The last
            # h-tile uses a small trailing m-chunk so the final copy+DMA
            # chain after the last matmul is short.
            def d_group(ht, parts):
                d_ps = [
                    psD.tile([P, ln], f32, tag=f"d{jj}", name=f"d{jj}_{ht}")
                    for jj, (m0, ln) in enumerate(parts)
                ]
                for it in range(NI):
                    wslice = wd_sb[:, it, ht * P:(ht + 1) * P]
                    for jj, (m0, ln) in enumerate(parts):
                        nc.tensor.matmul(
                            d_ps[jj][:],
                            wslice,
                            acts[:, it, m0:m0 + ln],
                            start=(it == 0),
                            stop=(it == NI - 1),
                        )
                return d_ps

            def d_store(ht, jj, m0, ln, d_ps):
                y_sb = youts.tile([P, ln], bf16, tag=f"y{jj}", name=f"y{jj}_{ht}")
                if jj == 0:
                    nc.scalar.copy(y_sb[:], d_ps[:])
                    nc.sync.dma_start(y.ap()[ht, :, m0:m0 + ln], y_sb[:])
                else:
                    nc.vector.tensor_copy(y_sb[:], d_ps[:])
                    nc.scalar.dma_start(y.ap()[ht, :, m0:m0 + ln], y_sb[:])

            c3 = 3 * (M_PAD // 4)
            last_chunks = ((0, c3), (c3, M_PAD - c3))
            for ht in range(HT):
                parts = chunks if ht < HT - 1 else last_chunks
                d_ps = d_group(ht, parts)
                for jj, (m0, ln) in enumerate(parts):
                    d_store(ht, jj, m0, ln, d_ps[jj])

    nc.compile()
    return nc


def _pack_w(w_e):
    """[H, I] -> [NI, P, KH*P] bf16 with w[it, p, k*P+i] = w_e[k*P+p, it*P+i]."""
    w4 = w_e.reshape(KH, P, NI, P)
    return np.ascontiguousarray(
        w4.transpose(2, 1, 0, 3).reshape(NI, P, KH * P).astype(BF16)
    )


def kernel(hidden_states, gate_w, wg, wu, wd):
    global LAST_RESULTS
    x = np.ascontiguousarray(np.asarray(hidden_states, dtype=np.float32))
    gw = np.asarray(gate_w, dtype=np.float32)
    wg = np.asarray(wg, dtype=np.float32)
    wu = np.asarray(wu, dtype=np.float32)
    wd = np.asarray(wd, dtype=np.float32)

    # Router on host (part of the dispatch/sharding step).
    logits = np.tanh((x @ gw) / np.float32(SOFTCAP))
    top2 = np.argsort(-logits, axis=1, kind="stable")[:, :TOPK]  # [T, 2]
    v = np.take_along_axis(logits, top2, axis=1)                 # descending
    ex = np.exp(v - v[:, :1])
    pk = (ex / ex.sum(axis=1, keepdims=True)).astype(np.float32)  # [T, 2]

    token_ids, probs_e = [], []
    for e in range(E):
        mask = top2 == e
        rows = np.where(mask.any(axis=1))[0]
        kk = np.argmax(mask[rows], axis=1)
        token_ids.append(rows)
        probs_e.append(pk[rows, kk])

    n_max = max(len(r) for r in token_ids)
    M_PAD = max(64, -(-n_max // 16) * 16)  # both m-chunks equal multiples of 8

    nc = _compiled.get(M_PAD)
    if nc is None:
        nc = _build(M_PAD)
        _compiled[M_PAD] = nc

    in_maps = []
    for e in range(E):
        ids = token_ids[e]
        xe = np.zeros((M_PAD, H), np.float32)
        xe[: len(ids)] = x[ids]
        # [M_PAD, KH, P] -> [KH, P, M_PAD]
        xt_e = np.ascontiguousarray(
            xe.reshape(M_PAD, KH, P).transpose(1, 2, 0).astype(BF16)
        )
        in_maps.append(
            {
                "xt": xt_e,
                "wg": _pack_w(wg[e]),
                "wu": _pack_w(wu[e]),
                "wd": np.ascontiguousarray(wd[e].astype(BF16)),
            }
        )

    res = run_bass_kernel_spmd(nc, in_maps, core_ids=list(range(N_CORES)))
    LAST_RESULTS = res

    out = np.zeros((T, H), np.float32)
    for e in range(E):
        ids = token_ids[e]
        y_e = res.results[e]["y"].reshape(H, M_PAD).astype(np.float32)
        out[ids] += probs_e[e][:, None] * y_e[:, : len(ids)].T
    return out
